# revision 1
# baseline (speedup 1.0000x reference)
"""DeepFM forward (embedding_lookup) on 8 Trainium2 NeuronCores.

Sharding: data-parallel over batch (16384 -> 8 x 2048), embedding table +
MLP weights replicated per core (256MB table fits easily in HBM), so no
collectives are needed.

Per-core dataflow:
  - one indirect-DMA gather per 128-sample tile fetches 66 rows/sample
    (26 onehot + 2x20 multihot) from a combined bf16 table [1M, 68]
    (cols 0:64 = fm_emb, col 64 = fm_w, 65:68 pad) -> G [128, 66, 68]
  - multihot means, FM 1st/2nd order stats via DVE reduces + ACT
    Square(accum_out=...)
  - PE transposes assemble xT [1792+13 features, batch] in bf16
  - 3-layer MLP as bf16 matmuls (f32 PSUM accumulation), bias+ReLU fused
    on the scalar engine, final sigmoid fused with the FM term via the
    activation bias input.
"""

import numpy as np
import ml_dtypes

import concourse.bass as bass
import concourse.mybir as mybir
import concourse.tile as tile
from concourse import bacc
from concourse.bass import IndirectOffsetOnAxis
from concourse.bass_utils import run_bass_kernel_spmd
from concourse.masks import make_identity

BF16 = mybir.dt.bfloat16
F32 = mybir.dt.float32
I32 = mybir.dt.int32

NCORES = 8
BATCH = 16384
BC = BATCH // NCORES          # samples per core
P = 128                       # partitions / tile size
NT = BC // P                  # sample tiles per core (16)
GT = 4                        # tiles per group (batch 512 for matmul N)
NG = NT // GT                 # groups per core (4)
NB = GT * P                   # batch per group (512)

ONEHOT = 26
MH = 2
MHL = 20
SLOTS = ONEHOT + MH * MHL     # gather slots per sample (66)
EMB = 64
TW = 68                       # table row width (64 emb + fm_w + 3 pad)
FEAT = EMB * (ONEHOT + MH)    # 1792 emb features
DENSE = 13
KC = 15                       # k chunks for layer 1 (1792 + 13 -> 1920)
U0, U1 = 1024, 512
M0 = U0 // P                  # 8 m-tiles layer 1
M1 = U1 // P                  # 4 m-tiles layer 2
K1 = U0 // P                  # 8 k-chunks layer 2
K2 = U1 // P                  # 4 k-chunks layer 3

AX = mybir.AxisListType
ALU = mybir.AluOpType
ACTF = mybir.ActivationFunctionType


def build_program(loop_n: int = 1):
    nc = bacc.Bacc("TRN2", target_bir_lowering=False, debug=False,
                   num_devices=NCORES)

    tbl = nc.dram_tensor("tbl", [1_000_000 * TW, 1], BF16, kind="ExternalInput")
    offs_d = nc.dram_tensor("offs", [P, NT, SLOTS], I32, kind="ExternalInput")
    denseT_d = nc.dram_tensor("denseT", [P, BC], BF16, kind="ExternalInput")
    w0_d = nc.dram_tensor("w0", [P, KC, U0], BF16, kind="ExternalInput")
    w1_d = nc.dram_tensor("w1", [P, K1, U1], BF16, kind="ExternalInput")
    w2_d = nc.dram_tensor("w2", [P, K2, 1], BF16, kind="ExternalInput")
    b0_d = nc.dram_tensor("b0", [P, 1], F32, kind="ExternalInput")
    b1_d = nc.dram_tensor("b1", [P, 1], F32, kind="ExternalInput")
    b2_d = nc.dram_tensor("b2", [P, 1], F32, kind="ExternalInput")
    out_d = nc.dram_tensor("out", [NT, P], F32, kind="ExternalOutput")

    with tile.TileContext(nc) as tc:
        with (
            tc.tile_pool(name="persist", bufs=1) as pp,
            tc.tile_pool(name="gather", bufs=4) as gp,
            tc.tile_pool(name="fm", bufs=2) as fp,
            tc.tile_pool(name="xt", bufs=2) as xp,
            tc.tile_pool(name="act", bufs=2) as hp,
            tc.tile_pool(name="pt", bufs=3, space="PSUM") as ptp,
            tc.tile_pool(name="mm", bufs=2, space="PSUM") as mmp,
            tc.tile_pool(name="ps3", bufs=1, space="PSUM") as p3p,
        ):
            def body():
                _body(nc, tc, pp, gp, fp, xp, hp, ptp, mmp, p3p,
                      tbl, offs_d, denseT_d, w0_d, w1_d, w2_d,
                      b0_d, b1_d, b2_d, out_d)

            if loop_n > 1:
                with tc.For_i(0, loop_n, 1):
                    body()
            else:
                body()

    nc.compile()
    return nc


def _body(nc, tc, pp, gp, fp, xp, hp, ptp, mmp, p3p,
          tbl, offs_d, denseT_d, w0_d, w1_d, w2_d, b0_d, b1_d, b2_d, out_d):
    if True:
        if True:
            # ---------- persistent tiles ----------
            ident = pp.tile([P, P], BF16)
            identf = pp.tile([P, P], F32)
            make_identity(nc, ident[:])
            make_identity(nc, identf[:])

            offs = pp.tile([P, NT, SLOTS], I32)
            w0 = pp.tile([P, KC, U0], BF16)
            w1 = pp.tile([P, K1, U1], BF16)
            w2 = pp.tile([P, K2, 1], BF16)
            b0 = pp.tile([P, 1], F32)
            b1 = pp.tile([P, 1], F32)
            b2 = pp.tile([P, 1], F32)
            fmc = pp.tile([P, NT], F32)   # per-sample FM contribution
            outf = pp.tile([P, NT], F32)  # sigmoid outputs, [p, t]

            nc.sync.dma_start(out=offs[:], in_=offs_d[:])
            nc.sync.dma_start(out=w0[:], in_=w0_d[:])
            nc.sync.dma_start(out=w1[:], in_=w1_d[:])
            nc.sync.dma_start(out=w2[:], in_=w2_d[:])
            nc.sync.dma_start(out=b0[:], in_=b0_d[:])
            nc.sync.dma_start(out=b1[:], in_=b1_d[:])
            nc.sync.dma_start(out=b2[:], in_=b2_d[:])

            for g in range(NG):
                xt = xp.tile([P, KC, NB], BF16, tag="xt")
                # dense features arrive pre-transposed/padded from host
                nc.sync.dma_start(out=xt[:, KC - 1:KC, :],
                                  in_=denseT_d[:, g * NB:(g + 1) * NB])

                for tt in range(GT):
                    t = g * GT + tt
                    # one [128,1]-offset indirect gather per slot (the only
                    # form walrus lowers correctly); multihot slots gather
                    # with CCE-add accumulation so the 20-way sum happens in
                    # the DMA engines.
                    gt = gp.tile([P, ONEHOT, TW], BF16, tag="g")
                    gt2d = gt[:].rearrange("p c w -> p (c w)")
                    offs2d = offs[:].rearrange("p t s -> p (t s)")
                    for c in range(ONEHOT):
                        o = t * SLOTS + c
                        nc.gpsimd.indirect_dma_start(
                            out=gt2d[:, c * TW:(c + 1) * TW],
                            out_offset=None,
                            in_=tbl[:],
                            in_offset=IndirectOffsetOnAxis(
                                ap=offs2d[:, o:o + 1], axis=0),
                        )
                    # fully independent multihot gathers (no CCE RMW chains);
                    # 64-wide dest (multihot needs no fm_w col -> 128B
                    # descriptors); the 20-way mean reduction runs on the
                    # vector engine, which has slack under the Pool stream
                    gmh = gp.tile([P, MH * MHL, EMB], BF16, tag="jacc")
                    gmh2d = gmh[:].rearrange("p c w -> p (c w)")
                    for f in range(MH):
                        for j in range(MHL):
                            o = t * SLOTS + ONEHOT + f * MHL + j
                            base = (f * MHL + j) * EMB
                            nc.gpsimd.indirect_dma_start(
                                out=gmh2d[:, base:base + EMB],
                                out_offset=None,
                                in_=tbl[:],
                                in_offset=IndirectOffsetOnAxis(
                                    ap=offs2d[:, o:o + 1], axis=0),
                            )
                    jsum = fp.tile([P, MH, EMB], F32, tag="jsum")
                    nc.vector.tensor_reduce(
                        out=jsum[:],
                        in_=gmh[:].rearrange(
                            "p (f j) e -> p f e j", f=MH),
                        axis=AX.X, op=ALU.add)

                    # dense cat_emb tile [p, 28 fields, 64]: onehot emb
                    # compacted (drop fm_w/pad cols) + multihot means
                    gc = gp.tile([P, ONEHOT + MH, EMB], BF16, tag="gc")
                    nc.scalar.activation(
                        out=gc[:, 0:ONEHOT, :],
                        in_=gt[:, 0:ONEHOT, 0:EMB],
                        func=ACTF.Copy)
                    nc.vector.tensor_scalar_mul(
                        out=gc[:, ONEHOT:ONEHOT + MH, :],
                        in0=jsum[:],
                        scalar1=1.0 / MHL)

                    # s = sum over all 28 fields of cat_emb, per emb dim
                    sbe = fp.tile([P, EMB], F32, tag="sbe")
                    nc.vector.tensor_reduce(
                        out=sbe[:],
                        in_=gc[:].rearrange("p f e -> p e f"),
                        axis=AX.X, op=ALU.add)

                    # sum of squares over all fields, and sum of s^2
                    sq_all = fp.tile([P, (ONEHOT + MH) * EMB], BF16,
                                     tag="sq_all")
                    acc_sq = fp.tile([P, 1], F32, tag="acc_sq")
                    nc.scalar.activation(
                        out=sq_all[:],
                        in_=gc[:].rearrange("p f e -> p (f e)"),
                        func=ACTF.Square, accum_out=acc_sq[:])
                    sq_s = fp.tile([P, EMB], F32, tag="sq_s")
                    acc_ss = fp.tile([P, 1], F32, tag="acc_ss")
                    nc.scalar.activation(out=sq_s[:], in_=sbe[:],
                                         func=ACTF.Square,
                                         accum_out=acc_ss[:])

                    # fm 1st order: sum of fm_w over onehot slots
                    fm1 = fp.tile([P, 1], F32, tag="fm1")
                    nc.vector.tensor_reduce(
                        out=fm1[:],
                        in_=gt[:, 0:ONEHOT, EMB:EMB + 1].rearrange(
                            "p f o -> p o f"),
                        axis=AX.X, op=ALU.add)

                    # fmc = fm1 + 0.5*(acc_ss - acc_sq)
                    tmp = fp.tile([P, 1], F32, tag="tmp")
                    nc.vector.tensor_tensor(out=tmp[:], in0=acc_ss[:],
                                            in1=acc_sq[:], op=ALU.subtract)
                    nc.vector.scalar_tensor_tensor(
                        out=fmc[:, t:t + 1], in0=tmp[:], scalar=0.5,
                        in1=fm1[:], op0=ALU.mult, op1=ALU.add)

                    # transposes: 14 feature chunks of 128 (pairs of fields)
                    for b4 in range(4):
                        n = 4 if b4 < 3 else 2
                        pt = ptp.tile([P, 512], BF16, tag="pt")
                        for j in range(n):
                            c = b4 * 4 + j
                            nc.tensor.transpose(
                                out=pt[:, j * P:(j + 1) * P],
                                in_=gc[:, 2 * c:2 * c + 2, :].rearrange(
                                    "p f e -> p (f e)"),
                                identity=ident[:])
                        nc.vector.tensor_copy(
                            out=xt[:, b4 * 4:b4 * 4 + n,
                                   tt * P:(tt + 1) * P],
                            in_=pt[:, 0:n * P].rearrange(
                                "p (c b) -> p c b", c=n))

                # ---------- MLP for this group ----------
                h0 = hp.tile([P, K1, NB], BF16, tag="h0")
                for mi in range(M0):
                    ps = mmp.tile([P, 512], F32, tag="mmps")
                    for c in range(KC):
                        nc.tensor.matmul(
                            out=ps[:],
                            lhsT=w0[:, c:c + 1, mi * P:(mi + 1) * P],
                            rhs=xt[:, c:c + 1, :],
                            start=(c == 0), stop=(c == KC - 1))
                    nc.scalar.activation(out=h0[:, mi, :], in_=ps[:],
                                         func=ACTF.Relu, bias=b0[:, :])

                h1 = hp.tile([P, K2, NB], BF16, tag="h1")
                for mi in range(M1):
                    ps = mmp.tile([P, 512], F32, tag="mmps")
                    for c in range(K1):
                        nc.tensor.matmul(
                            out=ps[:],
                            lhsT=w1[:, c:c + 1, mi * P:(mi + 1) * P],
                            rhs=h0[:, c:c + 1, :],
                            start=(c == 0), stop=(c == K1 - 1))
                    nc.scalar.activation(out=h1[:, mi, :], in_=ps[:],
                                         func=ACTF.Relu, bias=b1[:, :])

                ps3 = p3p.tile([P, GT], F32, tag="ps3")
                for tt in range(GT):
                    for c in range(K2):
                        nc.tensor.matmul(
                            out=ps3[:, tt:tt + 1],
                            lhsT=h1[:, c:c + 1, tt * P:(tt + 1) * P],
                            rhs=w2[:, c:c + 1, :],
                            start=(c == 0), stop=(c == K2 - 1))
                for tt in range(GT):
                    t = g * GT + tt
                    r3 = fp.tile([P, 1], F32, tag="r3")
                    nc.scalar.activation(out=r3[:], in_=ps3[:, tt:tt + 1],
                                         func=ACTF.Relu, bias=b2[:, :])
                    nc.scalar.activation(out=outf[:, t:t + 1], in_=r3[:],
                                         func=ACTF.Sigmoid,
                                         bias=fmc[:, t:t + 1])

            # ---------- final transpose + store ----------
            pso = p3p.tile([NT, P], F32, tag="pso")
            nc.tensor.transpose(out=pso[:], in_=outf[:], identity=identf[:])
            outt = pp.tile([NT, P], F32)
            nc.vector.tensor_copy(out=outt[:], in_=pso[:])
            nc.sync.dma_start(out=out_d[:], in_=outt[:])


def prepare_inputs(dense, onehot, multihot, fm_w, fm_emb,
                   w0, b0, w1, b1, w2, b2):
    """Host-side layout prep (casts / pads / transposes only)."""
    bf = ml_dtypes.bfloat16
    f32 = np.float32

    tbl = np.zeros((1_000_000, TW), dtype=bf)
    tbl[:, :EMB] = np.asarray(fm_emb, f32).astype(bf)
    tbl[:, EMB] = np.asarray(fm_w, f32).reshape(-1).astype(bf)

    w0p = np.zeros((KC * P, U0), dtype=bf)
    w0p[:FEAT + DENSE] = np.asarray(w0, f32).astype(bf)
    w0p = np.ascontiguousarray(
        w0p.reshape(KC, P, U0).transpose(1, 0, 2))
    w1p = np.ascontiguousarray(
        np.asarray(w1, f32).astype(bf).reshape(K1, P, U1).transpose(1, 0, 2))
    w2p = np.ascontiguousarray(
        np.asarray(w2, f32).astype(bf).reshape(K2, P, 1).transpose(1, 0, 2))
    b0r = np.full((P, 1), np.asarray(b0, f32).reshape(-1)[0], f32)
    b1r = np.full((P, 1), np.asarray(b1, f32).reshape(-1)[0], f32)
    b2r = np.full((P, 1), np.asarray(b2, f32).reshape(-1)[0], f32)

    onehot = np.asarray(onehot).astype(np.int32)
    multihot = np.asarray(multihot).astype(np.int32)
    dense = np.asarray(dense, f32)

    in_maps = []
    for c in range(NCORES):
        r0, r1 = c * BC, (c + 1) * BC
        oh = onehot[r0:r1].reshape(NT, P, ONEHOT).transpose(1, 0, 2)
        mh = multihot[r0:r1].reshape(NT, P, MH * MHL).transpose(1, 0, 2)
        offs = np.ascontiguousarray(
            np.concatenate([oh, mh], axis=2).astype(np.int32) * TW)
        dT = np.zeros((P, BC), dtype=bf)
        dT[:DENSE, :] = dense[r0:r1].T.astype(bf)
        in_maps.append({
            "tbl": tbl.reshape(-1, 1), "offs": offs, "denseT": dT,
            "w0": w0p, "w1": w1p, "w2": w2p,
            "b0": b0r, "b1": b1r, "b2": b2r,
        })
    return in_maps


_NC_CACHE = [None]


def kernel(**inputs) -> np.ndarray:
    in_maps = prepare_inputs(**inputs)
    if _NC_CACHE[0] is None:
        _NC_CACHE[0] = build_program()
    nc = _NC_CACHE[0]
    res = run_bass_kernel_spmd(nc, in_maps, core_ids=list(range(NCORES)))
    outs = [res.results[c]["out"].reshape(BC, 1) for c in range(NCORES)]
    return np.concatenate(outs, axis=0).astype(np.float32)


if __name__ == "__main__":
    import reference
    inputs = {k: np.asarray(v) for k, v in reference.setup_inputs().items()}
    expected = np.asarray(reference.reference(**inputs))
    actual = kernel(**inputs)
    err = np.abs(actual - expected)
    rel = np.abs(err) / (np.abs(expected) + 1e-9)
    print("max abs err", err.max(), "max rel err", rel.max(),
          "mean rel", rel.mean())

